# revision 1
# baseline (speedup 1.0000x reference)
"""Causal multi-head attention (B=4, T=2048, E=1024, H=16, D=64) on 8 TRN2 cores.

Sharding: core c = (batch b = c//2, parity gp = c%2). Each core computes all 16
heads for its batch, restricted to its interleaved half of the 128-row query
blocks (gp=0 -> even blocks, gp=1 -> odd blocks). K/V projections are duplicated
within a pair of cores, which avoids any cross-core collective.

A single SPMD program handles both parities: core gp=0 receives x shifted right
by 128 positions (zero-padded front), turning its even blocks into the odd
blocks of the shifted sequence. The zero-pad contributes exactly exp(0)*128 to
each softmax denominator, which is subtracted via a per-core constant.

Scores are computed transposed (S^T[tk, tq]) so the softmax needs no transposes:
exp without max-subtraction (scores are O(10) bounded), denominators from an
augmented ones-block in the V operand, normalization by reciprocal-multiply.
All matmuls run in float32r (relaxed fp32, full PE rate at free dim >= 256).
"""

import numpy as np

_B, _T, _E, _H, _D = 4, 2048, 1024, 16, 64
_NCORES = 8
_TB = 128          # t block
_NBLK = _T // _TB  # 16 global blocks
_MYT = _T // 2     # rows per core


def _build_nc(repeats=1):
    import concourse.mybir as mybir
    import concourse.tile as tile
    from concourse import bacc

    f32 = mybir.dt.float32
    f32r = mybir.dt.float32r
    bf16 = mybir.dt.bfloat16
    EXP = mybir.ActivationFunctionType.Exp
    ADD = mybir.AluOpType.add

    nc = bacc.Bacc("TRN2", target_bir_lowering=False, debug=False,
                   num_devices=_NCORES)

    xt_d = nc.dram_tensor("xt", [_E, _T], f32, kind="ExternalInput").ap()
    wk_d = nc.dram_tensor("wk", [_E, _E], f32, kind="ExternalInput").ap()
    wv_d = nc.dram_tensor("wv", [_E, _E], f32, kind="ExternalInput").ap()
    wq_d = nc.dram_tensor("wq", [_E, _E], f32, kind="ExternalInput").ap()
    wot_d = nc.dram_tensor("wot", [_E, _E], f32, kind="ExternalInput").ap()
    bo_d = nc.dram_tensor("bo_b", [128, _E], f32, kind="ExternalInput").ap()
    tri_d = nc.dram_tensor("tri", [128, 128], f32, kind="ExternalInput").ap()
    ones0_d = nc.dram_tensor("ones0", [128, 64], f32, kind="ExternalInput").ap()
    y_d = nc.dram_tensor("y", [_MYT, _E], f32, kind="ExternalOutput").ap()

    xt_r = xt_d.rearrange("(e p) t -> p e t", p=128)      # [128, 8, 2048]
    wk_r = wk_d.rearrange("(e p) m -> p e m", p=128)      # [128, 8, 1024]
    wv_r = wv_d.rearrange("(e p) m -> p e m", p=128)
    wq_r = wq_d.rearrange("(e p) m -> p e m", p=128)

    with tile.TileContext(nc) as tc:
        with (
            tc.tile_pool(name="big", bufs=1) as big,
            tc.tile_pool(name="strm", bufs=2) as strm,
            tc.tile_pool(name="ptp", bufs=2) as ptp,
            tc.tile_pool(name="sml", bufs=2) as sml,
            tc.tile_pool(name="nrm", bufs=2) as nrm,
            tc.tile_pool(name="ps", bufs=2, space="PSUM") as ps,
        ):
          for _rep in range(repeats):
              # constants
              tri_t = big.tile([128, 128], bf16, tag="tri")
              nc.gpsimd.dma_start(tri_t[:], tri_d)

              # x^T resident for the whole kernel: [128, 8 echunks, 2048]
              xt = big.tile([128, 8, _T], f32r, tag="xt")
              for e in range(8):
                  nc.gpsimd.dma_start(xt[:, e], xt_r[:, e])

              # attention outputs, one tile per head pair: [128 hd, 1024 myT]
              att = [big.tile([128, _MYT], f32r, tag=f"att{p}", name=f"att{p}") for p in range(8)]

              for qd in range(4):  # head quads: heads 4qd .. 4qd+3
                  # ---- load quad weights ----
                  wk_t = [strm.tile([128, 8, 128], f32r, tag="wk", name="wk_t") for _ in range(2)]
                  wq_t = [strm.tile([128, 8, 128], f32r, tag="wq", name="wq_t") for _ in range(2)]
                  for s2 in range(2):
                      col = 256 * qd + 128 * s2
                      nc.gpsimd.dma_start(wk_t[s2][:], wk_r[:, :, col:col + 128])
                      nc.gpsimd.dma_start(wq_t[s2][:], wq_r[:, :, col:col + 128])
                  wv_t = big.tile([128, 8, 256], f32r, tag="wv")
                  nc.gpsimd.dma_start(wv_t[:], wv_r[:, :, 256 * qd:256 * qd + 256])

                  # ---- projections ----
                  ktq = [big.tile([128, _T], f32r, tag=f"kt{s2}", name=f"ktq", bufs=2) for s2 in range(2)]
                  qtq = [big.tile([128, _MYT], f32r, tag=f"qt{s2}", name=f"qtq") for s2 in range(2)]
                  vaq = [big.tile([128, 512], bf16, tag=f"va{tb}", name=f"vaq") for tb in range(16)]

                  # kT: [128 hd, 2048 t] per pair slice
                  for s2 in range(2):
                      for tcc in range(4):
                          pk = ps.tile([128, 512], f32, tag="proj", bufs=2)
                          for e in range(8):
                              nc.tensor.matmul(
                                  pk[:], wk_t[s2][:, e],
                                  xt[:, e, 512 * tcc:512 * tcc + 512],
                                  start=(e == 0), stop=(e == 7))
                          nc.vector.tensor_copy(
                              ktq[s2][:, 512 * tcc:512 * tcc + 512], pk[:])

                  # qT: odd 128-blocks of xt only -> [128 hd, 1024 myT]
                  for s2 in range(2):
                      for stc in range(4):
                          pq = ps.tile([128, 512], f32, tag="proj", bufs=2)
                          rhs = xt[:, :, 512 * stc:512 * stc + 512].rearrange(
                              "p e (two n) -> p e two n", two=2)[:, :, :, 128:256]
                          for e in range(8):
                              nc.tensor.matmul(
                                  pq[:, 0:256], wq_t[s2][:, e], rhs[:, e],
                                  start=(e == 0), stop=(e == 7))
                          nc.vector.tensor_copy(
                              qtq[s2][:, 256 * stc:256 * stc + 256], pq[:, 0:256])

                  # v: per t-block [128 t, 384] = [v_even | ones | v_odd] per pair
                  for tb in range(16):
                      pv = ps.tile([128, 512], f32, tag="proj", bufs=2)
                      for e in range(8):
                          nc.tensor.matmul(
                              pv[:, 0:256], xt[:, e, 128 * tb:128 * tb + 128],
                              wv_t[:, e], start=(e == 0), stop=(e == 7))
                      src = pv[:, 0:256].rearrange(
                          "p (h x) -> p h x", h=4)
                      dst = vaq[tb][:].rearrange("p (h z) -> p h z", h=4)
                      nc.vector.tensor_copy(dst[:, :, 0:64], src[:])
                      if tb == 0:
                          # block 0 is the zero-pad on shifted cores: its ones
                          # column comes from input data (0 there, 1 elsewhere)
                          for h4 in range(4):
                              nc.gpsimd.dma_start(dst[:, h4, 64:128], ones0_d)
                      else:
                          nc.gpsimd.memset(dst[:, :, 64:128], 1.0)

                  # ---- attention ----
                  for s2 in range(2):         # pair in quad (outer: frees
                      for g in range(4):      # ktq[s2] at quad half-way)
                          p_idx = 2 * qd + s2
                          # both heads of the pair interleaved: their K=64 QK
                          # matmuls live in disjoint PE row groups (0-63/64-127)
                          # and execute concurrently
                          po2 = [ps.tile([128, 512], f32, tag="outp",
                                         name="po2", bufs=3) for _ in range(2)]
                          n_cp = 2 * (g + 1)  # chunk pairs; chunks 0..4g+3
                          for cp in range(n_cp):
                              sc2 = [ps.tile([128, 512], f32, tag="score",
                                             name="sc2", bufs=3) for _ in range(2)]
                              for q2 in range(2):
                                  c = 2 * cp + q2
                                  for hh in range(2):
                                      nc.tensor.matmul(
                                          sc2[hh][:, 256 * q2:256 * q2 + 256],
                                          ktq[s2][64 * hh:64 * hh + 64,
                                                  128 * c:128 * c + 128],
                                          qtq[s2][64 * hh:64 * hh + 64,
                                                  256 * g:256 * g + 256],
                                          start=True, stop=True)
                              for hh in range(2):
                                  pt = ptp.tile([128, 512], bf16, tag="pt", bufs=3)
                                  nc.scalar.activation(
                                      pt[:], sc2[hh][:], EXP, scale=0.125)
                                  if cp == n_cp - 2:
                                      # chunk 4g+1 = diag of block A
                                      nc.vector.tensor_mul(
                                          pt[:, 256:384],
                                          pt[:, 256:384], tri_t[:])
                                  if cp == n_cp - 1:
                                      # A-half of 4g+2, 4g+3 invalid;
                                      # chunk 4g+3 = diag of block B
                                      nc.gpsimd.memset(
                                          pt[:, 0:512].rearrange(
                                              "p (two x) -> p two x", two=2)
                                          [:, :, 0:128], 0.0)
                                      nc.vector.tensor_mul(
                                          pt[:, 384:512],
                                          pt[:, 384:512], tri_t[:])
                                  for q2 in range(2):
                                      c = 2 * cp + q2
                                      nc.tensor.matmul(
                                          po2[hh][:, 0:256],
                                          vaq[c][:, 128 * (2 * s2 + hh):
                                                 128 * (2 * s2 + hh) + 128],
                                          pt[:, 256 * q2:256 * q2 + 256],
                                          start=(cp == 0 and q2 == 0),
                                          stop=(cp == n_cp - 1 and q2 == 1))
                          for hh in range(2):
                              po = po2[hh]
                              sums_rows = po[64:128, 0:256]
                              v_rows = po[0:64, 0:256]
                              sums_t = nrm.tile([64, 256], f32, tag="sums")
                              nc.vector.tensor_copy(sums_t[:], sums_rows)
                              rec_t = nrm.tile([64, 256], f32, tag="rec")
                              nc.vector.reciprocal_approx_fast(rec_t[:], sums_t[:])
                              nc.vector.tensor_mul(
                                  att[p_idx][64 * hh:64 * hh + 64,
                                             256 * g:256 * g + 256],
                                  v_rows, rec_t[:])

              # ---- output projection: y = att @ wo^T + bo ----
              for ec in range(2):
                  bo_t = big.tile([128, 512], f32, tag="bo", name="bo_t")
                  nc.sync.dma_start(bo_t[:], bo_d[:, 512 * ec:512 * ec + 512])
                  wot_t = big.tile([128, 8, 512], f32r, tag="wot", name="wot_t")
                  nc.gpsimd.dma_start(
                      wot_t[:],
                      wot_d.rearrange("(p pp) e -> pp p e", pp=128)
                      [:, :, 512 * ec:512 * ec + 512])
                  for tb in range(8):
                      py = ps.tile([128, 512], f32, tag="proj", bufs=2)
                      for p in range(8):
                          nc.tensor.matmul(
                              py[:], att[p][:, 128 * tb:128 * tb + 128],
                              wot_t[:, p], start=(p == 0), stop=(p == 7))
                      ysb = sml.tile([128, 512], f32, tag="ysb", bufs=1)
                      nc.vector.tensor_add(
                          ysb[:], py[:], bo_t[:])
                      nc.sync.dma_start(
                          y_d[128 * tb:128 * tb + 128, 512 * ec:512 * ec + 512],
                          ysb[:])

    nc.compile()
    return nc


_NC_CACHE = {}


def _get_nc(repeats=1):
    if repeats not in _NC_CACHE:
        _NC_CACHE[repeats] = _build_nc(repeats)
    return _NC_CACHE[repeats]


def _make_in_maps(x, wq, wk, wv, wo, bo):
    x = np.asarray(x, dtype=np.float32)
    wq = np.asarray(wq, dtype=np.float32)
    wk = np.asarray(wk, dtype=np.float32)
    wv = np.asarray(wv, dtype=np.float32)
    wo = np.asarray(wo, dtype=np.float32)
    bo = np.asarray(bo, dtype=np.float32)

    # [H, E, D] -> [E, H*D]
    wq2 = np.ascontiguousarray(wq.transpose(1, 0, 2).reshape(_E, _H * _D))
    wk2 = np.ascontiguousarray(wk.transpose(1, 0, 2).reshape(_E, _H * _D))
    wv2 = np.ascontiguousarray(wv.transpose(1, 0, 2).reshape(_E, _H * _D))
    wot = np.ascontiguousarray(wo.T)                       # [hd, e_out]
    bo_b = np.ascontiguousarray(np.broadcast_to(bo, (128, _E)))
    tri = np.ascontiguousarray(
        np.triu(np.ones((128, 128), dtype=np.float32)))    # tk <= tq

    in_maps = []
    for c in range(_NCORES):
        b, gp = c // 2, c % 2
        xt = np.ascontiguousarray(x[b].T)                  # [E, T]
        if gp == 0:
            xt_s = np.zeros_like(xt)
            xt_s[:, _TB:] = xt[:, :_T - _TB]
            xt = xt_s
            ones0 = np.zeros((128, 64), dtype=np.float32)
        else:
            ones0 = np.ones((128, 64), dtype=np.float32)
        in_maps.append({
            "xt": np.ascontiguousarray(xt), "wk": wk2, "wv": wv2, "wq": wq2,
            "wot": wot, "bo_b": bo_b, "tri": tri, "ones0": ones0,
        })
    return in_maps


def kernel(x, wq, wk, wv, wo, bo, _want_results=False, _repeats=1, **_ignored):
    from concourse.bass_utils import run_bass_kernel_spmd

    nc = _get_nc(_repeats)
    in_maps = _make_in_maps(x, wq, wk, wv, wo, bo)
    res = run_bass_kernel_spmd(nc, in_maps, core_ids=list(range(_NCORES)))

    out = np.empty((_B, _T, _E), dtype=np.float32)
    for c in range(_NCORES):
        b, gp = c // 2, c % 2
        yc = res.results[c]["y"].reshape(_NBLK // 2, _TB, _E)
        out[b].reshape(_NBLK, _TB, _E)[gp::2] = yc
    if _want_results:
        return out, res
    return out



# revision 11
# speedup vs baseline: 3.6529x; 3.6529x over previous
"""Causal multi-head attention (B=4, T=2048, E=1024, H=16, D=64) on 8 TRN2 cores.

Sharding: core c = (batch b = c//2, parity gp = c%2). Each core computes all 16
heads for its batch, restricted to its interleaved half of the 128-row query
blocks (gp=0 -> even blocks, gp=1 -> odd blocks). K/V projections are duplicated
within a pair of cores, which avoids any cross-core collective.

A single SPMD program handles both parities: core gp=0 receives x shifted right
by 128 positions (zero-padded front), turning its even blocks into the odd
blocks of the shifted sequence. The zero-pad contributes exactly exp(0)*128 to
each softmax denominator, which is subtracted via a per-core constant.

Scores are computed transposed (S^T[tk, tq]) so the softmax needs no transposes:
exp without max-subtraction (scores are O(10) bounded), denominators from an
augmented ones-block in the V operand, normalization by reciprocal-multiply.
All matmuls run in float32r (relaxed fp32, full PE rate at free dim >= 256).
"""

import numpy as np

_B, _T, _E, _H, _D = 4, 2048, 1024, 16, 64
_NCORES = 8
_TB = 128          # t block
_NBLK = _T // _TB  # 16 global blocks
_MYT = _T // 2     # rows per core


def _build_nc(repeats=1):
    import concourse.mybir as mybir
    import concourse.tile as tile
    from concourse import bacc

    f32 = mybir.dt.float32
    f32r = mybir.dt.float32r
    bf16 = mybir.dt.bfloat16
    EXP = mybir.ActivationFunctionType.Exp
    ADD = mybir.AluOpType.add

    nc = bacc.Bacc("TRN2", target_bir_lowering=False, debug=False,
                   num_devices=_NCORES)

    xt_d = nc.dram_tensor("xt", [_E, _T], f32, kind="ExternalInput").ap()
    wk_d = nc.dram_tensor("wk", [_E, _E], f32, kind="ExternalInput").ap()
    wv_d = nc.dram_tensor("wv", [_E, _E], f32, kind="ExternalInput").ap()
    wq_d = nc.dram_tensor("wq", [_E, _E], f32, kind="ExternalInput").ap()
    wot_d = nc.dram_tensor("wot", [_E, _E], f32r, kind="ExternalInput").ap()
    bo_d = nc.dram_tensor("bo_b", [128, _E], f32, kind="ExternalInput").ap()
    tri_d = nc.dram_tensor("tri", [128, 128], f32, kind="ExternalInput").ap()
    ones0_d = nc.dram_tensor("ones0", [128, 64], f32, kind="ExternalInput").ap()
    y_d = nc.dram_tensor("y", [_MYT, _E], f32, kind="ExternalOutput").ap()

    xt_r = xt_d.rearrange("(e p) t -> p e t", p=128)      # [128, 8, 2048]
    wk_r = wk_d.rearrange("(e p) m -> p e m", p=128)      # [128, 8, 1024]
    wv_r = wv_d.rearrange("(e p) m -> p e m", p=128)
    wq_r = wq_d.rearrange("(e p) m -> p e m", p=128)

    with tile.TileContext(nc) as tc:
        with (
            tc.tile_pool(name="big", bufs=1) as big,
            tc.tile_pool(name="strm", bufs=2) as strm,
            tc.tile_pool(name="ptp", bufs=2) as ptp,
            tc.tile_pool(name="sml", bufs=2) as sml,
            tc.tile_pool(name="nrm", bufs=2) as nrm,
            tc.tile_pool(name="ps", bufs=2, space="PSUM") as ps,
        ):
          for _rep in range(repeats):
              # x^T resident for the whole kernel: [128, 8 echunks, 2048],
              # DMA'd in 512-column blocks so the first projection tile only
              # waits for block 0
              xt = big.tile([128, 8, _T], f32r, tag="xt")

              def xt_dma(cb):
                  nc.gpsimd.dma_start(xt[:, :, 512 * cb:512 * cb + 512],
                                      xt_r[:, :, 512 * cb:512 * cb + 512])

              def load_quad_weights(qd, first=False):
                  # DMA-queue order matters at startup: K weights, then the
                  # first x^T block (so the K projection starts ~5us in),
                  # then the rest interleaved by first-use order.
                  wk_t = [strm.tile([128, 8, 128], f32r, tag="wk", name="wk_t") for _ in range(2)]
                  wq_t = [strm.tile([128, 8, 128], f32r, tag="wq", name="wq_t") for _ in range(2)]
                  for s2 in range(2):
                      col = 256 * qd + 128 * s2
                      nc.gpsimd.dma_start(wk_t[s2][:], wk_r[:, :, col:col + 128])
                  if first:
                      xt_dma(0)
                  for s2 in range(2):
                      col = 256 * qd + 128 * s2
                      nc.gpsimd.dma_start(wq_t[s2][:], wq_r[:, :, col:col + 128])
                  if first:
                      xt_dma(1)
                  wv_t = big.tile([128, 8, 256], f32r, tag="wv")
                  nc.gpsimd.dma_start(wv_t[:], wv_r[:, :, 256 * qd:256 * qd + 256])
                  if first:
                      xt_dma(2)
                      xt_dma(3)
                  return wk_t, wq_t, wv_t

              # quad-0 weights + x^T, interleaved
              wtiles = load_quad_weights(0, first=True)

              # constants (small; casting DMAs must run on gpsimd)
              tri_t = big.tile([128, 128], bf16, tag="tri")
              nc.gpsimd.dma_start(tri_t[:], tri_d)

              # output-projection weights: prefetched well before they're used
              wot_t = big.tile([128, 8, _E], bf16, tag="wot", name="wot_t")
              bo_t = big.tile([128, _E], f32, tag="bo", name="bo_t")

              # attention outputs, one tile per head pair: [128 hd, 1024 myT]
              att = [big.tile([128, _MYT], bf16, tag=f"att{p}", name=f"att{p}") for p in range(8)]

              for qd in range(4):  # head quads: heads 4qd .. 4qd+3
                  wk_t, wq_t, wv_t = wtiles

                  # ---- projections ----
                  ktq = [big.tile([128, _T], f32r, tag=f"kt{s2}", name=f"ktq", bufs=2) for s2 in range(2)]
                  qtq = [big.tile([128, _MYT], f32r, tag=f"qt{s2}", name=f"qtq") for s2 in range(2)]
                  vaq = [big.tile([128, 512], bf16, tag=f"va{tb}", name=f"vaq") for tb in range(16)]

                  # kT: [128 hd, 2048 t] per pair slice
                  for s2 in range(2):
                      for tcc in range(4):
                          pk = ps.tile([128, 512], f32, tag="proj", bufs=2)
                          for e in range(8):
                              nc.tensor.matmul(
                                  pk[:], wk_t[s2][:, e],
                                  xt[:, e, 512 * tcc:512 * tcc + 512],
                                  start=(e == 0), stop=(e == 7))
                          nc.vector.tensor_copy(
                              ktq[s2][:, 512 * tcc:512 * tcc + 512], pk[:])

                  # qT: odd 128-blocks of xt only -> [128 hd, 1024 myT]
                  for s2 in range(2):
                      for stc in range(4):
                          pq = ps.tile([128, 512], f32, tag="proj", bufs=2)
                          rhs = xt[:, :, 512 * stc:512 * stc + 512].rearrange(
                              "p e (two n) -> p e two n", two=2)[:, :, :, 128:256]
                          for e in range(8):
                              nc.tensor.matmul(
                                  pq[:, 0:256], wq_t[s2][:, e], rhs[:, e],
                                  start=(e == 0), stop=(e == 7))
                          nc.vector.tensor_copy(
                              qtq[s2][:, 256 * stc:256 * stc + 256], pq[:, 0:256])

                  # v: per t-block [128 t, 384] = [v_even | ones | v_odd] per pair
                  for tb in range(16):
                      pv = ps.tile([128, 512], f32, tag="proj", bufs=2)
                      for e in range(8):
                          nc.tensor.matmul(
                              pv[:, 0:256], xt[:, e, 128 * tb:128 * tb + 128],
                              wv_t[:, e], start=(e == 0), stop=(e == 7))
                      src = pv[:, 0:256].rearrange(
                          "p (h x) -> p h x", h=4)
                      dst = vaq[tb][:].rearrange("p (h z) -> p h z", h=4)
                      nc.vector.tensor_copy(dst[:, :, 0:64], src[:])
                      if qd == 0:
                          # ones columns are identical across quads: write once
                          if tb == 0:
                              # block 0 is the zero-pad on shifted cores: its
                              # ones column comes from input data (0 there, 1
                              # elsewhere)
                              for h4 in range(4):
                                  nc.gpsimd.dma_start(dst[:, h4, 64:128], ones0_d)
                          else:
                              nc.gpsimd.memset(dst[:, :, 64:128], 1.0)

                  # prefetch next quad's weights; they land during attention
                  if qd < 3:
                      wtiles = load_quad_weights(qd + 1)
                  if qd == 1:
                      # output-projection weights arrive during quads 2-3
                      # (gpsimd: the f32 -> bf16 casting DMA requires it)
                      nc.sync.dma_start(bo_t[:], bo_d[:, :])
                      nc.gpsimd.dma_start(
                          wot_t[:], wot_d.rearrange("(p pp) e -> pp p e", pp=128))

                  # ---- attention ----
                  for s2 in range(2):         # pair in quad (outer: frees
                      for g in range(4):      # ktq[s2] at quad half-way)
                          p_idx = 2 * qd + s2
                          # both heads of the pair interleaved: their K=64 QK
                          # matmuls live in disjoint PE row groups (0-63/64-127)
                          # and execute concurrently
                          po2 = [ps.tile([128, 512], f32, tag="outp",
                                         name="po2", bufs=3) for _ in range(2)]
                          n_cp = 2 * (g + 1)  # chunk pairs; chunks 0..4g+3
                          for cp in range(n_cp):
                              sc2 = [ps.tile([128, 512], f32, tag="score",
                                             name="sc2", bufs=3) for _ in range(2)]
                              for q2 in range(2):
                                  c = 2 * cp + q2
                                  for hh in range(2):
                                      nc.tensor.matmul(
                                          sc2[hh][:, 256 * q2:256 * q2 + 256],
                                          ktq[s2][64 * hh:64 * hh + 64,
                                                  128 * c:128 * c + 128],
                                          qtq[s2][64 * hh:64 * hh + 64,
                                                  256 * g:256 * g + 256],
                                          start=True, stop=True)
                              for hh in range(2):
                                  pt = ptp.tile([128, 512], bf16, tag="pt", bufs=3)
                                  nc.scalar.activation(
                                      pt[:], sc2[hh][:], EXP, scale=0.125)
                                  if cp == n_cp - 2:
                                      # chunk 4g+1 = diag of block A
                                      nc.vector.tensor_mul(
                                          pt[:, 256:384],
                                          pt[:, 256:384], tri_t[:])
                                  if cp == n_cp - 1:
                                      # A-half of 4g+2, 4g+3 invalid;
                                      # chunk 4g+3 = diag of block B
                                      nc.gpsimd.memset(
                                          pt[:, 0:512].rearrange(
                                              "p (two x) -> p two x", two=2)
                                          [:, :, 0:128], 0.0)
                                      nc.vector.tensor_mul(
                                          pt[:, 384:512],
                                          pt[:, 384:512], tri_t[:])
                                  for q2 in range(2):
                                      c = 2 * cp + q2
                                      nc.tensor.matmul(
                                          po2[hh][:, 0:256],
                                          vaq[c][:, 128 * (2 * s2 + hh):
                                                 128 * (2 * s2 + hh) + 128],
                                          pt[:, 256 * q2:256 * q2 + 256],
                                          start=(cp == 0 and q2 == 0),
                                          stop=(cp == n_cp - 1 and q2 == 1))
                          for hh in range(2):
                              po = po2[hh]
                              sums_rows = po[64:128, 0:256]
                              v_rows = po[0:64, 0:256]
                              sums_t = nrm.tile([64, 256], f32, tag="sums")
                              nc.vector.tensor_copy(sums_t[:], sums_rows)
                              rec_t = nrm.tile([64, 256], f32, tag="rec")
                              nc.vector.reciprocal_approx_fast(rec_t[:], sums_t[:])
                              nc.vector.tensor_mul(
                                  att[p_idx][64 * hh:64 * hh + 64,
                                             256 * g:256 * g + 256],
                                  v_rows, rec_t[:])

              # ---- output projection: y = att @ wo^T + bo ----
              for ec in range(2):
                  for tb in range(8):
                      py = ps.tile([128, 512], f32, tag="proj", bufs=2)
                      for p in range(8):
                          nc.tensor.matmul(
                              py[:], att[p][:, 128 * tb:128 * tb + 128],
                              wot_t[:, p, 512 * ec:512 * ec + 512],
                              start=(p == 0), stop=(p == 7))
                      ysb = sml.tile([128, 512], f32, tag="ysb", bufs=2)
                      nc.vector.tensor_add(
                          ysb[:], py[:], bo_t[:, 512 * ec:512 * ec + 512])
                      nc.sync.dma_start(
                          y_d[128 * tb:128 * tb + 128, 512 * ec:512 * ec + 512],
                          ysb[:])

    nc.compile()
    return nc


_NC_CACHE = {}


def _get_nc(repeats=1):
    if repeats not in _NC_CACHE:
        _NC_CACHE[repeats] = _build_nc(repeats)
    return _NC_CACHE[repeats]


def _make_in_maps(x, wq, wk, wv, wo, bo):
    x = np.asarray(x, dtype=np.float32)
    wq = np.asarray(wq, dtype=np.float32)
    wk = np.asarray(wk, dtype=np.float32)
    wv = np.asarray(wv, dtype=np.float32)
    wo = np.asarray(wo, dtype=np.float32)
    bo = np.asarray(bo, dtype=np.float32)

    # [H, E, D] -> [E, H*D]
    wq2 = np.ascontiguousarray(wq.transpose(1, 0, 2).reshape(_E, _H * _D))
    wk2 = np.ascontiguousarray(wk.transpose(1, 0, 2).reshape(_E, _H * _D))
    wv2 = np.ascontiguousarray(wv.transpose(1, 0, 2).reshape(_E, _H * _D))
    wot = np.ascontiguousarray(wo.T)                       # [hd, e_out]
    bo_b = np.ascontiguousarray(np.broadcast_to(bo, (128, _E)))
    tri = np.ascontiguousarray(
        np.triu(np.ones((128, 128), dtype=np.float32)))    # tk <= tq

    in_maps = []
    for c in range(_NCORES):
        b, gp = c // 2, c % 2
        xt = np.ascontiguousarray(x[b].T)                  # [E, T]
        if gp == 0:
            xt_s = np.zeros_like(xt)
            xt_s[:, _TB:] = xt[:, :_T - _TB]
            xt = xt_s
            ones0 = np.zeros((128, 64), dtype=np.float32)
        else:
            ones0 = np.ones((128, 64), dtype=np.float32)
        in_maps.append({
            "xt": np.ascontiguousarray(xt), "wk": wk2, "wv": wv2, "wq": wq2,
            "wot": wot, "bo_b": bo_b, "tri": tri, "ones0": ones0,
        })
    return in_maps


def kernel(x, wq, wk, wv, wo, bo, _want_results=False, _repeats=1, **_ignored):
    from concourse.bass_utils import run_bass_kernel_spmd

    nc = _get_nc(_repeats)
    in_maps = _make_in_maps(x, wq, wk, wv, wo, bo)
    res = run_bass_kernel_spmd(nc, in_maps, core_ids=list(range(_NCORES)))

    out = np.empty((_B, _T, _E), dtype=np.float32)
    for c in range(_NCORES):
        b, gp = c // 2, c % 2
        yc = res.results[c]["y"].reshape(_NBLK // 2, _TB, _E)
        out[b].reshape(_NBLK, _TB, _E)[gp::2] = yc
    if _want_results:
        return out, res
    return out

